# revision 71
# baseline (speedup 1.0000x reference)
"""CrossModalAttention Trainium2 kernel, v4.

Schedule: 16 (si, a) score slots per frame; all other work (prev-frame
attend/out-proj, next-frame load/proj, sums/recips) is interleaved into
the slots at fine grain so no engine's in-order queue blocks long.
Scores psum is 4-deep so the score->min->sigmoid pipeline overlaps
across head-pairs.  Per-si-half DMA transposes; the last frame runs its
attention s-granular to shorten the tail.

Data-parallel over B*T = 32 frames -> 4 frames per core on 8 cores.

Dtype strategy (validated vs reference on host, rel_err ~5e-3):
 - Q/K/V projections: fp8e4m3 DoubleRow matmuls with a scaled-residual
   weight split: psum = fp8(64*W)^T fp8(x) + fp8(1024*dW)^T fp8(x/16),
   evac scales by 1/64.  2x PE throughput vs fp16.
 - scores: fp16 Q,K (1 cyc/row).
 - sigmoid: Act engine, psum f32 -> SBUF fp8 p2, bias = row min (DVE/Pool).
 - attend: fp8 DoubleRow.  p2 fp8 pairs are transposed as fp16 units
   (DMA transpose has no 1-byte mode); the implied k-pairing permutation
   is absorbed into the kT evacuation AP, so scores/sigmoid/attend all
   see one consistent internal k order.
 - attention row sums: PE matmuls vs a ones fp8 vector (frees Act from
   accum reads); 1/sum applied during the attend-psum evac as a fused
   tensor_tensor multiply with a DMA-broadcast reciprocal tile.
 - out-proj: fp16, evacuated to fp16 SBUF and DMA-stored.
bv/bo/bk/temporal_sync drop out or are applied on the host (bv@Wo + bo).
"""

import math

import numpy as np

import concourse.bass as bass
import concourse.bacc as bacc
import concourse.mybir as mybir
import concourse.tile as tile
from concourse import bass_utils

F8 = mybir.dt.float8e4
F16 = mybir.dt.float16
F32 = mybir.dt.float32
AF = mybir.ActivationFunctionType
ALU = mybir.AluOpType
DR = mybir.MatmulPerfMode.DoubleRow

B, T, S, D = 2, 16, 512, 512
H, HD = 8, 64
NCORES = 8
FRAMES = B * T // NCORES  # 4 frames per core
NT = D // 128
ALPHA = 64.0


def _emit(tc, nc, aps):
    qkv, wall, wo, bq64, out = aps

    with tc.tile_pool(name="wpool", bufs=1) as wpool, \
         tc.tile_pool(name="tpool", bufs=4) as tpool, \
         tc.tile_pool(name="projpool", bufs=2) as projpool, \
         tc.tile_pool(name="ppool", bufs=2) as ppool, \
         tc.tile_pool(name="apool", bufs=2) as apool, \
         tc.tile_pool(name="statpool", bufs=24) as statpool, \
         tc.tile_pool(name="rpool", bufs=2) as rpool, \
         tc.tile_pool(name="sps", bufs=4, space="PSUM") as sps, \
         tc.tile_pool(name="mmps", bufs=2, space="PSUM") as mmps, \
         tc.tile_pool(name="attps", bufs=1, space="PSUM") as attps, \
         tc.tile_pool(name="smps", bufs=1, space="PSUM") as smps:

        # ---------------- weights (one-time) -------------
        wall_sb = wpool.tile([128, 2, 3, NT, 512], F8)   # [p, res, w, dblk, n]
        wo_sb = wpool.tile([128, NT, 512], F16)          # [p, dblk, n]
        bq_sb = wpool.tile([128, NT], F32)               # 64*bq5, [p, j]
        ones8 = wpool.tile([128, 2, 64], F8)
        ones_p = wpool.tile([128, 64], F8)

        def load_weights_qk():
            nc.gpsimd.dma_start(wall_sb[:, :, 0:2, :, :], wall[:, :, 0:2, :, :])
            nc.gpsimd.dma_start(bq_sb[:], bq64.rearrange("(i p) -> p i", p=128))
            nc.vector.memset(ones8[:], 1.0)
            nc.vector.memset(ones_p[:], 1.0)

        def load_weights_rest():
            nc.gpsimd.dma_start(wall_sb[:, :, 2, :, :], wall[:, :, 2, :, :])

        def load_wo():
            nc.gpsimd.dma_start(wo_sb[:], wo[:])

        # ---------------- per-frame state ----------------
        def alloc_state(f):
            st = {}
            # [dp, res, tensor, dblk, s]
            st["x8"] = tpool.tile([128, 2, 3, NT, 512], F8, tag="x8",
                                  name=f"x8_{f}")
            st["qT"] = projpool.tile([128, NT, 512], F16, tag="qT",
                                     name=f"qT_{f}")
            st["kT"] = projpool.tile([128, NT, 512], F16, tag="kT",
                                     name=f"kT_{f}")
            # V in fp8: [k%128, b, j, dout]  (k = 2*(b*128 + kp) + j)
            st["vN8"] = projpool.tile([128, 2, 2, 512], F8, tag="vN8", bufs=3,
                                      name=f"vN8_{f}")
            return st

        def emit_load(f, st):
            nc.gpsimd.dma_start(st["x8"][:], qkv[f])

        def emit_load_split(f, st):
            for t in range(3):
                nc.gpsimd.dma_start(st["x8"][:, :, t, :, :], qkv[f][:, :, t, :, :])

        def emit_proj_qk_j(st, j):
            for t in range(2):
                ps = mmps.tile([128, 512], F32, tag="mm",
                               name=f"mm{'qk'[t]}_{j}")
                for sh in range(2):
                    n = 0
                    for r in range(2):
                        for c in range(2):
                            rhs = st["x8"][:, r, t, 2 * c:2 * c + 2,
                                           256 * sh:256 * sh + 256]
                            nc.tensor.matmul(
                                ps[:, 256 * sh:256 * sh + 256],
                                wall_sb[:, r, t, 2 * c:2 * c + 2,
                                        128 * j:128 * j + 128],
                                rhs, start=(n == 0), stop=(n == 3),
                                perf_mode=DR)
                            n += 1
                if t == 0:
                    # qT = psum/64 + bq5  (GPSIMD cannot read PSUM: use Act)
                    nc.scalar.activation(
                        st["qT"][:, j, :], ps[:], AF.Identity,
                        bias=bq_sb[:, j:j + 1], scale=1.0 / ALPHA)
                else:
                    # kT = psum/64, with the k-pair permutation folded into
                    # the output AP: kappa = 256b + 2u + jj <- s = 256b+128jj+u
                    kout = st["kT"][:, j, :].rearrange(
                        "p (b u jj) -> p b u jj", b=2, jj=2)
                    kout = kout.transpose([0, 1, 3, 2])  # [p, b, jj, u]
                    pin = ps[:].rearrange("p (b jj u) -> p b jj u", b=2, jj=2)
                    nc.vector.tensor_scalar(
                        kout, pin, 1.0 / ALPHA, None, ALU.mult)

        def emit_proj_qk(st):
            for j in range(NT):
                emit_proj_qk_j(st, j)

        def emit_proj_v_m(st, m):
                ps = mmps.tile([128, 512], F32, tag="mm", name=f"mmv_{m}")
                for dh in range(2):
                    n = 0
                    for r in range(2):
                        for c in range(2):
                            lhsT = st["x8"][:, r, 2, 2 * c:2 * c + 2,
                                           128 * m:128 * m + 128]
                            rhs = wall_sb[:, r, 2, 2 * c:2 * c + 2,
                                          256 * dh:256 * dh + 256]
                            nc.tensor.matmul(
                                ps[:, 256 * dh:256 * dh + 256], lhsT, rhs,
                                start=(n == 0), stop=(n == 3), perf_mode=DR)
                            n += 1
                nc.scalar.activation(
                    st["vN8"][:, m // 2, m % 2, :], ps[:], AF.Copy,
                    scale=1.0 / ALPHA)

        # ---------------- attention ----------------
        def emit_scores(f, st, p2, si, a):
            m5 = statpool.tile([128, 2], F32, tag="m5",
                               name=f"m5_{f}_{si}_{a}")
            for jh, h in enumerate((2 * a, 2 * a + 1)):
                lo = 64 * (h % 2)
                s_ps = sps.tile([128, 512], F32, tag="s",
                                name=f"s_{f}_{si}_{h}")
                nc.tensor.matmul(
                    s_ps[:],
                    st["qT"][lo:lo + 64, a, 128 * si:128 * si + 128],
                    st["kT"][lo:lo + 64, a, :], start=True, stop=True)
                nc.vector.tensor_reduce(m5[:, jh:jh + 1], s_ps[:],
                                        mybir.AxisListType.X, ALU.min)
                nc.scalar.activation(p2[:, si % 2, h, :], s_ps[:],
                                     AF.Sigmoid, bias=m5[:, jh:jh + 1],
                                     scale=-1.0)

        def attn_rhs(attnT, h, b, s):
            a8 = attnT[:].bitcast(F8)       # [128, 2, 16, 256]
            sl = a8[:, s, 2 * h + b, :]     # [128, 256 (2q+j)]
            return sl.rearrange("p (q j) -> p j q", j=2)

        def attn_rhs_j(attnT, h, b, s, j):
            a8 = attnT[:].bitcast(F8)       # [128, 2, 16, 256]
            return a8[:, s, 2 * h + b, j::2]  # [128, 128]

        def emit_transpose(f, g, p2, attnT, s):
            # transpose one si-half of fp8 pairs as fp16 units
            p2v = p2[:].bitcast(F16)        # [128, 2, 8, 256]
            nc.sync.dma_start(attnT[:, s, :, :], p2v[:, s, :, :],
                              transpose=True)

        def emit_sums_bank(f, g, attnT, sm_tiles, bank, s_list=(0, 1)):
            # ones lhsT has M=64: head h=2a+e lands replicated on
            # partitions 64e..64e+63 -- exactly the partitions the attend
            # psum uses for that head, so recip output feeds the evac TT
            # directly (no broadcast).  Bank = a-pair; a%2 picks free half.
            # Only the first (ai=0, s=0) group may zero each partition row.
            if bank not in sm_tiles:
                sm_tiles[bank] = smps.tile([128, 512], F32, tag="sm",
                                           name=f"sm_{f}_{g}_{bank}")
            sm = sm_tiles[bank]
            for ai in range(2):
                a = 2 * bank + ai
                for e in range(2):
                    h = 2 * a + e
                    for s in s_list:
                        first = (ai == 0 and s == min(s_list))
                        o = sm[64 * e:64 * e + 64,
                               256 * ai + 128 * s:256 * ai + 128 * s + 128]
                        if e == 0:
                            # DoubleRow (dst partition must be 0)
                            for b in range(2):
                                nc.tensor.matmul(
                                    o, ones8[:], attn_rhs(attnT, h, b, s),
                                    start=(b == 0 and first and s == 0),
                                    stop=(b == 1),
                                    perf_mode=DR,
                                    tile_position=(0, 0),
                                    skip_group_check=(not first or s != 0))
                        else:
                            # plain fp8 at position 64
                            n = 0
                            for b in range(2):
                                for j in range(2):
                                    nc.tensor.matmul(
                                        o, ones_p[:],
                                        attn_rhs_j(attnT, h, b, s, j),
                                        start=(n == 0 and first and s == 0),
                                        stop=(n == 3),
                                        tile_position=(0, 64),
                                        skip_group_check=(not first or s != 0))
                                    n += 1

        def emit_recip_bank(f, g, sm_tiles, rsi, bank):
            with nc.allow_low_precision(reason="1/sum f16 vs fp8 p"):
                nc.vector.reciprocal(
                    rsi[bank][:],
                    sm_tiles[bank][:].rearrange("p (ai q) -> p ai q", ai=2))

        def emit_recip_bank_s(f, g, sm_tiles, rsi, bank, s):
            with nc.allow_low_precision(reason="1/sum f16 vs fp8 p"):
                nc.vector.reciprocal(
                    rsi[bank][:, :, 128 * s:128 * s + 128],
                    sm_tiles[bank][:].rearrange(
                        "p (ai q) -> p ai q", ai=2)[:, :, 128 * s:128 * s + 128])

        def emit_attend_a(f, st, attnT, aT, rsi, g, a):
                a_ps = attps.tile([128, 256], F32, tag="att",
                                  name=f"aps_{f}_{g}_{a}")
                for jh, h in enumerate((2 * a, 2 * a + 1)):
                    lo = 64 * jh
                    for s in range(2):
                        if jh == 0:
                            for b in range(2):
                                nc.tensor.matmul(
                                    a_ps[lo:lo + 64, 128 * s:128 * s + 128],
                                    st["vN8"][:, b, :, 64 * h:64 * h + 64],
                                    attn_rhs(attnT, h, b, s),
                                    start=(b == 0), stop=(b == 1),
                                    perf_mode=DR, tile_position=(0, 0))
                        else:
                            n = 0
                            for b in range(2):
                                for j in range(2):
                                    nc.tensor.matmul(
                                        a_ps[lo:lo + 64, 128 * s:128 * s + 128],
                                        st["vN8"][:, b, j, 64 * h:64 * h + 64],
                                        attn_rhs_j(attnT, h, b, s, j),
                                        start=(n == 0), stop=(n == 3),
                                        tile_position=(0, lo))
                                    n += 1
                dst = aT[:, a, 256 * g:256 * g + 256]
                nc.vector.tensor_copy(dst, a_ps[:])
                nc.gpsimd.tensor_tensor(dst, dst,
                                        rsi[a // 2][:, a % 2, :], ALU.mult)

        def emit_attend_a_s(f, st, attnT, aT, rsi, g, a, s):
                a_ps = attps.tile([128, 256], F32, tag="att",
                                  name=f"apss_{f}_{g}_{a}_{s}")
                a_ps = a_ps[:, 0:128]
                for jh, h in enumerate((2 * a, 2 * a + 1)):
                    lo = 64 * jh
                    if jh == 0:
                        for b in range(2):
                            nc.tensor.matmul(
                                a_ps[lo:lo + 64, :],
                                st["vN8"][:, b, :, 64 * h:64 * h + 64],
                                attn_rhs(attnT, h, b, s),
                                start=(b == 0), stop=(b == 1),
                                perf_mode=DR, tile_position=(0, 0))
                    else:
                        n = 0
                        for b in range(2):
                            for j in range(2):
                                nc.tensor.matmul(
                                    a_ps[lo:lo + 64, :],
                                    st["vN8"][:, b, j, 64 * h:64 * h + 64],
                                    attn_rhs_j(attnT, h, b, s, j),
                                    start=(n == 0), stop=(n == 3),
                                    tile_position=(0, lo))
                                n += 1
                dst = aT[:, a, 256 * g + 128 * s:256 * g + 128 * s + 128]
                if a % 2 == 0:
                    nc.scalar.activation(dst, a_ps[:], AF.Copy)
                else:
                    nc.vector.tensor_copy(dst, a_ps[:])
                nc.gpsimd.tensor_tensor(
                    dst, dst, rsi[a // 2][:, a % 2, 128 * s:128 * s + 128],
                    ALU.mult)

        def emit_outproj(f, aT, outsb, stp, store=False):
            ps = mmps.tile([128, 512], F32, tag="mm", name=f"mmo_{f}_{stp}")
            for jj in range(NT):
                nc.tensor.matmul(
                    ps[:], aT[:, jj, 128 * stp:128 * stp + 128],
                    wo_sb[:, jj, :], start=(jj == 0), stop=(jj == NT - 1))
            nc.scalar.activation(outsb[:, stp, :], ps[:], AF.Copy)
            if store:
                nc.gpsimd.dma_start(
                    out[f].rearrange("(a p) d -> p a d", p=128), outsb[:])

        def emit_store_part(f, outsb, lo, hi):
            nc.gpsimd.dma_start(
                out[f, 128 * lo:128 * hi, :].rearrange(
                    "(a p) d -> p a d", p=128),
                outsb[:, lo:hi, :])

        # ---------------- schedule ----------------
        st0 = alloc_state(0)
        load_weights_qk()
        nc.gpsimd.dma_start(st0["x8"][:, :, 0, :, :], qkv[0][:, :, 0, :, :])
        nc.gpsimd.dma_start(st0["x8"][:, :, 1, :, :], qkv[0][:, :, 1, :, :])
        load_weights_rest()
        nc.gpsimd.dma_start(st0["x8"][:, :, 2, :, :], qkv[0][:, :, 2, :, :])
        load_wo()
        warm = wpool.tile([1, 2], F16)
        nc.vector.memset(warm[:], 0.0)
        nc.scalar.activation(warm[:], warm[:], AF.Sigmoid)

        sts = [st0] + [alloc_state(f) for f in range(1, FRAMES)]
        st = st0
        prev = None
        carry = None  # previous frame's g1 sums/recips thunks
        for f in range(FRAMES):
            nxt = sts[f + 1] if f + 1 < FRAMES else None
            p2 = [ppool.tile([128, 2, H, 512], F8, tag=f"p{g}",
                             name=f"p_{f}_{g}") for g in range(2)]
            attnTs = [ppool.tile([128, 2, 16, 128], F16, tag=f"attnT{g}",
                                 name=f"attnT_{f}_{g}") for g in range(2)]
            rsis = [[rpool.tile([128, 2, 256], F16, tag=f"rsi{g}{bk}",
                                name=f"rsi_{f}_{g}_{bk}") for bk in range(2)]
                    for g in range(2)]
            aT = apool.tile([128, NT, 512], F16, tag="aT", name=f"aT_{f}")
            outsb = apool.tile([128, NT, 512], F16, tag="outsb",
                               name=f"outsb_{f}")
            sm_g = {0: {}, 1: {}}
            slots = [[] for _ in range(16)]

            def at(i, fn):
                slots[i].append(fn)

            if carry is not None:
                cs0, cs1, cr0, cr1 = carry
                at(3, cs0)
                at(4, cr0)
                at(5, cs1)
                at(6, cr1)
            if prev is not None:
                fp, stp_, attnTsp, aTp, outsbp, rsisp = prev
                for a in range(4):
                    at(a, (lambda fp=fp, stp_=stp_, attnTsp=attnTsp, aTp=aTp,
                           rsisp=rsisp, a=a: emit_attend_a(
                               fp, stp_, attnTsp[0], aTp, rsisp[0], 0, a)))
                at(4, (lambda fp=fp, aTp=aTp, outsbp=outsbp:
                       emit_outproj(fp, aTp, outsbp, 0)))
                at(6, (lambda fp=fp, aTp=aTp, outsbp=outsbp:
                       emit_outproj(fp, aTp, outsbp, 1)))
                for a in range(4):
                    at(7 + a, (lambda fp=fp, stp_=stp_, attnTsp=attnTsp,
                               aTp=aTp, rsisp=rsisp, a=a: emit_attend_a(
                                   fp, stp_, attnTsp[1], aTp, rsisp[1], 1, a)))
                at(11, (lambda fp=fp, aTp=aTp, outsbp=outsbp:
                        emit_outproj(fp, aTp, outsbp, 2)))
                at(12, (lambda fp=fp, aTp=aTp, outsbp=outsbp:
                        emit_outproj(fp, aTp, outsbp, 3, store=True)))
            if f == 0:
                for m in range(NT):
                    at(m, (lambda m=m: emit_proj_v_m(st0, m)))
            if nxt:
                if f == 0:
                    for t in range(3):
                        at(t, (lambda f=f, nxt=nxt, t=t: nc.gpsimd.dma_start(
                            nxt["x8"][:, :, t, :, :],
                            qkv[f + 1][:, :, t, :, :])))
                    for j in range(NT):
                        at(5 + j, (lambda nxt=nxt, j=j:
                                   emit_proj_qk_j(nxt, j)))
                    for m in range(NT):
                        at(9 + m, (lambda nxt=nxt, m=m:
                                   emit_proj_v_m(nxt, m)))
                else:
                    for j in range(NT):
                        at(13 + (j // 2), (lambda nxt=nxt, j=j:
                                           emit_proj_qk_j(nxt, j)))
                    for m in range(NT):
                        at(15, (lambda nxt=nxt, m=m: emit_proj_v_m(nxt, m)))
            if f + 2 < FRAMES:
                # prefetch input two frames ahead, away from the boundary
                at(13, (lambda f=f, s2=sts[f + 2]:
                        emit_load(f + 2, s2)))
            # this frame's g0 sums mid-si2 (its si0/si1 transposes are
            # triggered at the si boundaries), recips one slot later
            if nxt is not None:
                at(11, (lambda f=f, t=attnTs, s=sm_g:
                        emit_sums_bank(f, 0, t[0], s[0], 0)))
                at(12, (lambda f=f, s=sm_g, r=rsis:
                        emit_recip_bank(f, 0, s[0], r[0], 0)))
                at(13, (lambda f=f, t=attnTs, s=sm_g:
                        emit_sums_bank(f, 0, t[0], s[0], 1)))
                at(14, (lambda f=f, s=sm_g, r=rsis:
                        emit_recip_bank(f, 0, s[0], r[0], 1)))
            else:
                # last frame: s-granular g0 chain starting right after the
                # si0-half transpose lands
                at(5, (lambda f=f, t=attnTs, s=sm_g: (
                    emit_sums_bank(f, 0, t[0], s[0], 0, (0,)),
                    emit_sums_bank(f, 0, t[0], s[0], 1, (0,)))))
                at(6, (lambda f=f, s=sm_g, r=rsis: (
                    emit_recip_bank_s(f, 0, s[0], r[0], 0, 0),
                    emit_recip_bank_s(f, 0, s[0], r[0], 1, 0))))
                for a in range(4):
                    at(7 + a, (lambda f=f, st=st, t=attnTs, aT=aT, r=rsis,
                               a=a: emit_attend_a_s(f, st, t[0], aT, r[0],
                                                    0, a, 0)))
                at(11, (lambda f=f, aT=aT, outsb=outsb:
                        emit_outproj(f, aT, outsb, 0)))
                at(10, (lambda f=f, t=attnTs, s=sm_g: (
                    emit_sums_bank(f, 0, t[0], s[0], 0, (1,)),
                    emit_sums_bank(f, 0, t[0], s[0], 1, (1,)))))
                at(11, (lambda f=f, s=sm_g, r=rsis: (
                    emit_recip_bank_s(f, 0, s[0], r[0], 0, 1),
                    emit_recip_bank_s(f, 0, s[0], r[0], 1, 1))))
                for a in range(4):
                    at(12 + a, (lambda f=f, st=st, t=attnTs, aT=aT, r=rsis,
                                a=a: emit_attend_a_s(f, st, t[0], aT, r[0],
                                                     0, a, 1)))

            for si in range(NT):
                for a in range(4):
                    if f == 0 and si == 0:
                        emit_proj_qk_j(st, a)
                    emit_scores(f, st, p2[si // 2], si, a)
                    for fn in slots[si * 4 + a]:
                        fn()
                emit_transpose(f, si // 2, p2[si // 2], attnTs[si // 2],
                               si % 2)
            carry = (
                lambda f=f, t=attnTs, s=sm_g: emit_sums_bank(f, 1, t[1], s[1], 0),
                lambda f=f, t=attnTs, s=sm_g: emit_sums_bank(f, 1, t[1], s[1], 1),
                lambda f=f, s=sm_g, r=rsis: emit_recip_bank(f, 1, s[1], r[1], 0),
                lambda f=f, s=sm_g, r=rsis: emit_recip_bank(f, 1, s[1], r[1], 1),
            )
            if nxt is None:
                # tail: finish g0 out-proj, then the g1 chain s-granular
                emit_outproj(f, aT, outsb, 1)
                emit_sums_bank(f, 1, attnTs[1], sm_g[1], 0, (0,))
                emit_sums_bank(f, 1, attnTs[1], sm_g[1], 1, (0,))
                emit_recip_bank_s(f, 1, sm_g[1], rsis[1], 0, 0)
                emit_recip_bank_s(f, 1, sm_g[1], rsis[1], 1, 0)
                for a in range(4):
                    emit_attend_a_s(f, st, attnTs[1], aT, rsis[1], 1, a, 0)
                emit_outproj(f, aT, outsb, 2)
                emit_store_part(f, outsb, 0, 3)
                emit_sums_bank(f, 1, attnTs[1], sm_g[1], 0, (1,))
                emit_sums_bank(f, 1, attnTs[1], sm_g[1], 1, (1,))
                emit_recip_bank_s(f, 1, sm_g[1], rsis[1], 0, 1)
                emit_recip_bank_s(f, 1, sm_g[1], rsis[1], 1, 1)
                for a in range(4):
                    emit_attend_a_s(f, st, attnTs[1], aT, rsis[1], 1, a, 1)
                emit_outproj(f, aT, outsb, 3)
                emit_store_part(f, outsb, 3, 4)
            prev = (f, st, attnTs, aT, outsb, rsis)
            st = nxt


def build_nc():
    nc = bacc.Bacc("TRN2", target_bir_lowering=False, debug=False,
                   num_devices=NCORES)
    qkv = nc.dram_tensor("qkv", (FRAMES, 128, 2, 3, NT, 512), F8,
                         kind="ExternalInput").ap()
    wall = nc.dram_tensor("wall", (128, 2, 3, NT, D), F8,
                          kind="ExternalInput").ap()
    wo = nc.dram_tensor("wo", (128, NT, D), F16, kind="ExternalInput").ap()
    bq64 = nc.dram_tensor("bq64", (D,), F32, kind="ExternalInput").ap()
    out = nc.dram_tensor("out", (FRAMES, S, D), F16,
                         kind="ExternalOutput").ap()
    with tile.TileContext(nc) as tc:
        _emit(tc, nc, (qkv, wall, wo, bq64, out))
    nc.compile()
    return nc


_NC = None


def _get_nc():
    global _NC
    if _NC is None:
        _NC = build_nc()
    return _NC


def _f8(a):
    import ml_dtypes
    return np.asarray(a, np.float32).astype(ml_dtypes.float8_e4m3)


def make_in_maps(query_spikes, key_spikes, value_spikes, Wq, bq, Wk, bk,
                 Wv, bv, Wo, bo, modality_weights, temporal_sync,
                 query_modality, key_modality):
    qm = int(query_modality)
    km = int(key_modality)
    mw = np.asarray(modality_weights, np.float32)
    c = (mw[qm] * mw[km]) / np.float32(math.sqrt(HD))  # [H]
    scale_cols = np.repeat(-5.0 * c, HD).astype(np.float32)  # [D]
    wq_s = np.asarray(Wq, np.float32) * scale_cols[None, :]
    bq64 = (np.asarray(bq, np.float32) * scale_cols).astype(np.float32)

    # wall[p, r, w, i, n]: r0 = fp8(64*W), r1 = fp8(1024*(W - W8/64))
    def packw(w):
        w = np.asarray(w, np.float32)
        w8 = _f8(ALPHA * w)
        dw = ALPHA * w - w8.astype(np.float32)
        d8 = _f8(16.0 * dw)
        return w8, d8

    wq8, wqd = packw(wq_s)
    wk8, wkd = packw(Wk)
    wv8, wvd = packw(Wv)
    wall = np.stack([np.stack([wq8, wk8, wv8]),
                     np.stack([wqd, wkd, wvd])])  # [2r, 3w, D, D]
    wall = np.ascontiguousarray(
        wall.reshape(2, 3, NT, 128, D).transpose(3, 0, 1, 2, 4))
    wo16 = np.asarray(Wo, np.float32).astype(np.float16)
    wo16 = np.ascontiguousarray(
        wo16.reshape(NT, 128, D).transpose(1, 0, 2))
    shared = {"wall": wall, "wo": wo16, "bq64": bq64}

    # qkv[f, dp, r, st, t, db, sl]: r0 = fp8(x), r1 = fp8(x/16)
    x = np.stack([
        np.asarray(query_spikes, np.float32).reshape(B * T, S, D),
        np.asarray(key_spikes, np.float32).reshape(B * T, S, D),
        np.asarray(value_spikes, np.float32).reshape(B * T, S, D),
    ], axis=1)  # [F, 3, S, D]
    x8 = _f8(x)
    x8s = _f8(x / 16.0)
    qkv_all = np.stack([x8, x8s], axis=1)  # [F, 2r, 3t, S, D]
    # -> [f, dp, r, t, db, s]: element = x[f, t, s, db*128+dp]
    qkv_all = qkv_all.reshape(B * T, 2, 3, S, NT, 128).transpose(
        0, 5, 1, 2, 4, 3)
    in_maps = []
    for core in range(NCORES):
        sl = slice(core * FRAMES, (core + 1) * FRAMES)
        in_maps.append({
            "qkv": np.ascontiguousarray(qkv_all[sl]),
            **shared,
        })
    return in_maps


def host_bias(Wv_np, bv_np, Wo_np, bo_np):
    return (np.asarray(bv_np, np.float64) @ np.asarray(Wo_np, np.float64)
            + np.asarray(bo_np, np.float64)).astype(np.float32)


def kernel(**inputs):
    nc = _get_nc()
    in_maps = make_in_maps(**inputs)
    res = bass_utils.run_bass_kernel_spmd(
        nc, in_maps, core_ids=list(range(NCORES)))
    out = np.concatenate([np.asarray(r["out"], np.float16).astype(np.float32)
                          for r in res.results], axis=0)
    out += host_bias(inputs["Wv"], inputs["bv"], inputs["Wo"], inputs["bo"])
    return out.reshape(B, T, S, D)
